# revision 13
# baseline (speedup 1.0000x reference)
"""Trainium2 Bass kernel for nn_HSMM_4243427689001 (segment_reduce).

Math shortcut: the reference materializes scores[B,T,T,K] but the emission
output only reads the band start = end - (7-l), l in [0,8).  Moreover

    pre1[b,i,j] = (fwd[b,j+1]-fwd[b,i]) @ W1a + (bwd[b,i]-bwd[b,j+1]) @ W1b
                = N[b,j+1] - N[b,i],      N[b,t] = fwd[b,t] @ W1a - bwd[b,t] @ W1b

so the whole T x T x 2Q tensor collapses to a [B,T+1,Q] projection N plus
8 diagonal slices.  Per core (data parallel over batch, 4 sequences/core):
  1. indirect-DMA gather of the padded token embeddings, PE-transpose to
     x_embT [E=128, S*4]
  2. BiLSTM in transposed layout (h as [Q, batch]); input projections for all
     steps live in PSUM and the per-step W_hh matmul accumulates onto them
  3. N projection, tanh band strips, W2 matmul (with a ones-row to add b2),
     DMA of each diagonal strip straight to the output
"""

import numpy as np

B, T, V, E, Q, K, L = 32, 256, 30000, 128, 20, 50, 8
NCORES = 8
BL = B // NCORES          # 4 sequences per core
S = T + 2                 # 258 LSTM steps
NPOS = S * BL             # 1032 (step, batch) positions
NPOS_PAD = 1152           # 9 * 128 for the gather index tiles
G4 = 4 * Q                # 80 gate rows
GP = 116                  # padded gate rows: i@0, f@32, o@64, g@96 (quad-aligned)
SCH = 86                  # LSTM steps per PSUM chunk (3 chunks of 86)
NEG = -1e30
PAD1, PAD2 = 2, 3

_CACHE = {}


def _body(tc, aps):
    import concourse.bass as bass
    from concourse import mybir
    from concourse.masks import make_identity

    nc = tc.nc
    f32 = mybir.dt.float32
    AF = mybir.ActivationFunctionType
    OP = mybir.AluOpType
    MUL = OP.mult

    xt, emb, wihf, whhf, bf, wihb, whhb, bb, w1, b1, w2, b2, out = aps

    import contextlib

    _stack = contextlib.ExitStack()
    pers_pool = _stack.enter_context(tc.tile_pool(name="pers", bufs=1))

    def mktile(shape, dtype, name):
        return pers_pool.tile(shape, dtype, tag=name, name=name)

    # ---- constant / weight staging -------------------------------------
    ident = mktile([128, 128], f32, name="ident")
    make_identity(nc, ident[:])

    # gate permutation: torch cols i,f,g,o -> quad-aligned groups i@0 f@32 o@64
    # g@96 so every engine access starts at a partition multiple of 32
    def load_perm_w(dst, src):
        nc.vector.memset(dst[:], 0.0)  # pad columns must be 0, not NaN garbage
        nc.sync.dma_start(dst[:, 0:20], src[:, 0:20])
        nc.sync.dma_start(dst[:, 32:52], src[:, 20:40])
        nc.sync.dma_start(dst[:, 64:84], src[:, 60:80])
        nc.sync.dma_start(dst[:, 96:116], src[:, 40:60])

    wih_p = {}
    whh_p = {}
    bias_p = {}
    for d, (wih_s, whh_s, b_s) in (("f", (wihf, whhf, bf)), ("b", (wihb, whhb, bb))):
        wih_p[d] = mktile([E, GP], f32, name=f"wihp_{d}")
        load_perm_w(wih_p[d], wih_s)
        whh_p[d] = mktile([Q, GP], f32, name=f"whhp_{d}")
        load_perm_w(whh_p[d], whh_s)
        bias_p[d] = mktile([GP, 1], f32, name=f"biasp_{d}")
        nc.vector.memset(bias_p[d][:], 0.0)
        b2d = b_s.rearrange("(a b) -> a b", b=1)
        nc.sync.dma_start(bias_p[d][0:20, :], b2d[0:20, :])
        nc.sync.dma_start(bias_p[d][32:52, :], b2d[20:40, :])
        nc.sync.dma_start(bias_p[d][64:84, :], b2d[60:80, :])
        nc.sync.dma_start(bias_p[d][96:116, :], b2d[40:60, :])

    w1a = mktile([Q, Q], f32, name="w1a")
    nc.sync.dma_start(w1a[:, :], w1[0:Q, :])
    w1bn = mktile([Q, Q], f32, name="w1bn")
    nc.sync.dma_start(w1bn[:, :], w1[Q : 2 * Q, :])
    nc.scalar.mul(w1bn[:, :], w1bn[:, :], -1.0)  # N = fwd@W1a - bwd@W1b

    w2b = mktile([Q, K], f32, name="w2b")
    nc.sync.dma_start(w2b[:, :], w2[:, :])
    # b2 broadcast across all 128 partitions via a stride-0 DMA read
    b2bc = mktile([128, K], f32, name="b2bc")
    nc.sync.dma_start(b2bc[:, :], bass.AP(b2.tensor, 0, [[0, 128], [1, K]]))

    b1t = mktile([Q, 1], f32, name="b1t")
    nc.sync.dma_start(b1t[:, :], b1.rearrange("(a b) -> a b", b=1))

    neg_sb = mktile([(L - 1) * BL, K], f32, name="neg_sb")
    nc.vector.memset(neg_sb[:], NEG)

    xt_sb = mktile([128, 9], mybir.dt.int32, name="xt_sb")
    nc.sync.dma_start(xt_sb[:], xt.rearrange("(c p) -> p c", p=128))

    # ---- phase 1: embedding gather + transpose -------------------------
    xembT = mktile([E, NPOS], f32, name="xembT")  # col s*BL+b = emb[xp[b,s]]

    with (
        tc.tile_pool(name="gat", bufs=3) as gat_pool,
        tc.tile_pool(name="gpsum", bufs=6, space="PSUM") as gpool,
        tc.tile_pool(name="wpsum", bufs=2, space="PSUM") as wpool,
        tc.tile_pool(name="step", bufs=4) as spool,
        tc.tile_pool(name="band", bufs=2) as bpool,
    ):
        for c in range(9):
            n = 128 if c < 8 else NPOS - 8 * 128
            gt = gat_pool.tile([128, E], f32, tag="gt", name=f"gt{c}")
            nc.gpsimd.indirect_dma_start(
                out=gt[:n, :],
                out_offset=None,
                in_=emb[:, :],
                in_offset=bass.IndirectOffsetOnAxis(ap=xt_sb[:n, c : c + 1], axis=0),
            )
            tp = wpool.tile([128, 128], f32, tag="w", name=f"tp{c}")
            nc.tensor.transpose(tp[:, :n], gt[:n, :], ident[:n, :n])
            nc.vector.tensor_copy(xembT[:, c * 128 : c * 128 + n], tp[:, :n])

        # ---- phase 2: input projections into resident PSUM -------------
        G = {"f": [], "b": []}
        for d in ("f", "b"):
            for c in range(3):
                g = gpool.tile([GP, SCH * BL], f32, tag="g", name=f"g_{d}{c}")
                nc.tensor.matmul(
                    g[:, :],
                    lhsT=wih_p[d][:, :],
                    rhs=xembT[:, c * SCH * BL : (c + 1) * SCH * BL],
                    start=True,
                    stop=True,
                )
                G[d].append(g)

        # ---- phase 3: BiLSTM recurrence (transposed layout) -------------
        H = {
            "f": mktile([Q, NPOS], f32, name="HfT"),
            "b": mktile([Q, NPOS], f32, name="HbT"),
        }
        c_t = {
            "f": mktile([Q, BL], f32, name="c_f"),
            "b": mktile([Q, BL], f32, name="c_b"),
        }

        def lstm_step(d, s, s_prev):
            ch, off = divmod(s, SCH)
            off *= BL
            gsl = G[d][ch]
            if s_prev is not None:
                nc.tensor.matmul(
                    gsl[:, off : off + BL],
                    lhsT=whh_p[d][:, :],
                    rhs=H[d][:, s_prev * BL : (s_prev + 1) * BL],
                    start=False,
                    stop=True,
                    skip_group_check=True,
                )
            gs = gsl[:, off : off + BL]
            # one ACT per gate, every output at base partition 0 so all
            # DVE tensor_tensor ops see equal SB base partitions
            sigf = spool.tile([Q, BL], f32, tag="sigf", name=f"sf_{d}{s}")
            nc.scalar.activation(sigf[:], gs[32:52, :], AF.Sigmoid,
                                 bias=bias_p[d][32:52, :])
            sigi = spool.tile([Q, BL], f32, tag="sigi", name=f"si_{d}{s}")
            nc.scalar.activation(sigi[:], gs[0:20, :], AF.Sigmoid,
                                 bias=bias_p[d][0:20, :])
            tg = spool.tile([Q, BL], f32, tag="tg", name=f"tg_{d}{s}")
            nc.scalar.activation(tg[:], gs[96:116, :], AF.Tanh,
                                 bias=bias_p[d][96:116, :])
            sigo = spool.tile([Q, BL], f32, tag="sigo", name=f"so_{d}{s}")
            nc.scalar.activation(sigo[:], gs[64:84, :], AF.Sigmoid,
                                 bias=bias_p[d][64:84, :])
            if s_prev is None:
                nc.vector.tensor_tensor(c_t[d][:], sigi[:], tg[:], op=MUL)
            else:
                t1 = spool.tile([Q, BL], f32, tag="t1", name=f"t1_{d}{s}")
                nc.vector.tensor_tensor(t1[:], sigf[:], c_t[d][:], op=MUL)
                t2 = spool.tile([Q, BL], f32, tag="t2", name=f"t2_{d}{s}")
                nc.vector.tensor_tensor(t2[:], sigi[:], tg[:], op=MUL)
                nc.vector.tensor_add(c_t[d][:], t1[:], t2[:])
            th = spool.tile([Q, BL], f32, tag="th", name=f"th_{d}{s}")
            nc.scalar.activation(th[:], c_t[d][:], AF.Tanh)
            nc.vector.tensor_tensor(
                H[d][:, s * BL : (s + 1) * BL], sigo[:], th[:], op=MUL
            )

        for k in range(S):
            lstm_step("f", k, k - 1 if k else None)
            sb = S - 1 - k
            lstm_step("b", sb, sb + 1 if k else None)

        # ---- phase 4: N projection [Q, (T+1)*BL] ------------------------
        # N[:, t*BL+b] = W1a^T fwd[b,t] - W1b^T bwd[b,t];
        # fwd[b,t] = hf[t+1,b], bwd[b,t] = hb[t,b]
        NPT = (T + 1) * BL  # 1028
        NT_sb = mktile([Q, NPT], f32, name="NT_sb")
        for a, w in ((0, 344), (344, 344), (688, NPT - 688)):
            nps = wpool.tile([Q, 344], f32, tag="w", name=f"np{a}")
            nc.tensor.matmul(
                nps[:, :w], lhsT=w1a[:, :], rhs=H["f"][:, BL + a : BL + a + w],
                start=True, stop=False,
            )
            nc.tensor.matmul(
                nps[:, :w], lhsT=w1bn[:, :], rhs=H["b"][:, a : a + w],
                start=False, stop=True,
            )
            nc.vector.tensor_copy(NT_sb[:, a : a + w], nps[:, :w])

        # ---- phase 5: band scores + output ------------------------------
        for l in range(L):
            d = L - 1 - l
            n = (T - d) * BL
            pre = bpool.tile([Q, T * BL], f32, tag="pre", name=f"pre{l}")
            nc.vector.tensor_sub(
                pre[:, :n], NT_sb[:, (d + 1) * BL : (d + 1) * BL + n], NT_sb[:, :n]
            )
            hidp = bpool.tile([Q, T * BL], f32, tag="hidp", name=f"hidp{l}")
            nc.scalar.activation(hidp[:, :n], pre[:, :n], AF.Tanh, bias=b1t[:, :])
            for c in range((n + 127) // 128):
                w = min(128, n - c * 128)
                sc = wpool.tile([128, K], f32, tag="w", name=f"sc{l}_{c}")
                nc.tensor.matmul(
                    sc[:w, :], lhsT=hidp[:, c * 128 : c * 128 + w], rhs=w2b[:, :],
                    start=True, stop=True,
                )
                sc_sb = bpool.tile([128, K], f32, tag="sc_sb", name=f"scs{l}_{c}")
                nc.vector.tensor_add(sc_sb[:w, :], sc[:w, :], b2bc[:w, :])
                row0 = l * T * BL + d * BL + c * 128
                nc.sync.dma_start(out[row0 : row0 + w, :], sc_sb[:w, :])
            if d:
                nc.sync.dma_start(
                    out[l * T * BL : l * T * BL + d * BL, :], neg_sb[: d * BL, :]
                )


def _build_module():
    if "nc" in _CACHE:
        return _CACHE["nc"]
    import concourse.tile as tile
    from concourse import bacc, mybir

    f32 = mybir.dt.float32
    i32 = mybir.dt.int32

    nc = bacc.Bacc(
        "TRN2",
        target_bir_lowering=False,
        debug=False,
        enable_asserts=False,
        num_devices=NCORES,
    )
    xt = nc.dram_tensor("xt", [NPOS_PAD], i32, kind="ExternalInput").ap()
    emb = nc.dram_tensor("emb", [V, E], f32, kind="ExternalInput").ap()
    wihf = nc.dram_tensor("wihf", [E, G4], f32, kind="ExternalInput").ap()
    whhf = nc.dram_tensor("whhf", [Q, G4], f32, kind="ExternalInput").ap()
    bf = nc.dram_tensor("bf", [G4], f32, kind="ExternalInput").ap()
    wihb = nc.dram_tensor("wihb", [E, G4], f32, kind="ExternalInput").ap()
    whhb = nc.dram_tensor("whhb", [Q, G4], f32, kind="ExternalInput").ap()
    bb = nc.dram_tensor("bb", [G4], f32, kind="ExternalInput").ap()
    w1 = nc.dram_tensor("w1", [2 * Q, Q], f32, kind="ExternalInput").ap()
    b1 = nc.dram_tensor("b1", [Q], f32, kind="ExternalInput").ap()
    w2 = nc.dram_tensor("w2", [Q, K], f32, kind="ExternalInput").ap()
    b2 = nc.dram_tensor("b2", [K], f32, kind="ExternalInput").ap()
    out = (
        nc.dram_tensor("out", [L, T, BL, K], f32, kind="ExternalOutput")
        .ap()
        .rearrange("l t b k -> (l t b) k")
    )

    with tile.TileContext(nc) as tc:
        _body(tc, (xt, emb, wihf, whhf, bf, wihb, whhb, bb, w1, b1, w2, b2, out))
    nc.compile()
    _CACHE["nc"] = nc
    return nc


def _prep_in_maps(inputs):
    f = lambda k: np.ascontiguousarray(np.asarray(inputs[k], dtype=np.float32))
    x = np.asarray(inputs["x"]).astype(np.int32)  # [B, T]
    xp = np.concatenate(
        [np.full((B, 1), PAD1, np.int32), x, np.full((B, 1), PAD2, np.int32)], axis=1
    )  # [B, S]
    common = {
        "emb": f("emb"),
        "wihf": f("W_ih_f"),
        "whhf": f("W_hh_f"),
        "bf": f("b_f"),
        "wihb": f("W_ih_b"),
        "whhb": f("W_hh_b"),
        "bb": f("b_b"),
        "w1": f("W1"),
        "b1": f("b1"),
        "w2": f("W2"),
        "b2": f("b2"),
    }
    in_maps = []
    for r in range(NCORES):
        xt = np.zeros((NPOS_PAD,), np.int32)
        xt[:NPOS] = np.ascontiguousarray(xp[r * BL : (r + 1) * BL].T).reshape(-1)
        in_maps.append(dict(common, xt=xt))
    return in_maps


def _ensure_axon_hooks():
    """The image's antenv lacks axon_hooks; shim it so trace=True works."""
    import sys
    import types

    if "antenv.axon_hooks" in sys.modules:
        return
    import antenv

    mod = types.ModuleType("antenv.axon_hooks")
    mod._hook = None
    mod.set_axon_ntff_profile_hook = lambda h: setattr(mod, "_hook", h)
    mod.get_axon_ntff_profile_hook = lambda: mod._hook
    sys.modules["antenv.axon_hooks"] = mod
    antenv.axon_hooks = mod
    try:
        from trn_agent_boot.trn_boot import _ntff_profile_via_ctypes

        mod._hook = _ntff_profile_via_ctypes("/opt/axon/libaxon_pjrt.so")
    except Exception:
        pass


def _run(inputs, trace=False):
    if trace:
        _ensure_axon_hooks()
    from concourse.bass_utils import run_bass_kernel_spmd

    nc = _build_module()
    res = run_bass_kernel_spmd(
        nc, _prep_in_maps(inputs), core_ids=list(range(NCORES)), trace=trace
    )
    out = np.concatenate([r["out"] for r in res.results], axis=2)
    return out, res


def kernel(**inputs) -> np.ndarray:
    out, _ = _run(inputs)
    return out
